# revision 37
# baseline (speedup 1.0000x reference)
"""Trainium2 Bass kernel: causal self-attention (modded-nanogpt style),
tensor-parallel over heads across 8 NeuronCores with PHASED AllToAll
re-shards overlapped with attention compute.

Self-contained: hardcodes B=1, T=4096, D=1024, H=8, Hd=128, scale=0.12.

Per-core program (core = head). Query chunks of 512 rows are processed in
PAIRS (0,1)(2,3)(4,5)(6,7); within a pair the two chunks' S/AV matmuls are
interleaved so the PE pipeline never drains (TRN2 PE p-state ramps to full
clock only after ~3us of continuous execution). qkv-projection, q/k norm +
rope, and output-projection work for other chunks is emitted as FILLER
between attention steps to absorb exp-latency bubbles.

After each pair, that pair's 8 query blocks (128 rows each) are re-sharded
head->sequence with a small AllToAll (block 8k+j -> core j, slot layout
[8, 128, 128] fp16 = 256KB); 3 of the 4 collectives plus 3/4 of the output
projection are fully hidden under attention compute of later pairs.

Softmax denominator: exp tiles are accumulated on DVE (fp16), reduced over
the key axis with an all-ones [128,128] matmul (which also broadcasts the
row across all PSUM partitions), inverted with a single custom-DVE
reciprocal_approx_fast op, and multiplied into y^T.

Engine assignment: ACT = exp only; DVE = softmax accumulation, q/k scales,
batched rope, rsqrt/reciprocal magic; Pool/GpSimd = causal masks
(affine_select in place), PSUM evictions, sum-of-squares, v-mix, small
DMAs; PE = all matmuls; Sync = big DMAs.
"""

import os
import sys

sys.path.insert(0, "/opt/trn_rl_repo")

from contextlib import ExitStack

import numpy as np

import concourse.bass as bass
import concourse.bacc as bacc
import concourse.mybir as mybir
import concourse.tile as tile
from concourse.bass_utils import run_bass_kernel_spmd
from concourse.masks import make_identity

N_CORES = 8
T = 4096
D = 1024
H = 8
HD = 128
ATTN_SCALE = 0.12
P = 128
TCH = 512
NT = T // P          # 32 t-tiles (query/key blocks of 128)
NC_CH = T // TCH     # 8 chunks
NPAIR = NC_CH // 2   # 4 chunk pairs == 4 collectives
QUARTER = HD // 4

F32 = mybir.dt.float32
I32 = mybir.dt.int32
MMD = mybir.dt.float16
NP_MMD = np.float16
# exp(s - 12*ln2) = 2^-12 * exp(s): keeps fp16 exp values and their fp16
# partial sums in range; the scaling cancels in the softmax normalize.
EXP_BIAS = -8.317766166719343
RSQRT_MAGIC = 0x5F3759DF

_cached = {}


def build_module():
    nc = bacc.Bacc("TRN2", target_bir_lowering=False, debug=False,
                   num_devices=N_CORES)

    x_t = nc.dram_tensor("x_t", [D, T], MMD, kind="ExternalInput")
    w_qkv = nc.dram_tensor("w_qkv", [D, 3 * HD], MMD, kind="ExternalInput")
    # host-packed [p, tile, freq]/[p, tile, e] layouts -> one contiguous
    # DMA each (the natural [T, .] layouts DMA at 64B-element granularity)
    cos_t = nc.dram_tensor("cos_t", [P, NT * QUARTER], MMD,
                           kind="ExternalInput")
    sin_t = nc.dram_tensor("sin_t", [P, NT * QUARTER], MMD,
                           kind="ExternalInput")
    ve_h = nc.dram_tensor("ve_h", [P, NT * HD], MMD, kind="ExternalInput")
    lam = nc.dram_tensor("lam", [P, 2], F32, kind="ExternalInput")
    cpw = nc.dram_tensor("cpw", [D, D], MMD, kind="ExternalInput")
    # 4 row-blocks of 128: block (8k + core) lands at position k
    y_shard = nc.dram_tensor("y_shard", [4 * P, D], MMD,
                             kind="ExternalOutput")

    with tile.TileContext(nc) as tc, nc.allow_low_precision(
            reason="reduced-precision matmul operands"), ExitStack() as ctx:
        const = ctx.enter_context(tc.tile_pool(name="const", bufs=1))
        wqkv_pool = ctx.enter_context(tc.tile_pool(name="wqkv", bufs=1))
        big = ctx.enter_context(tc.tile_pool(name="big", bufs=1))
        xt_pool = ctx.enter_context(tc.tile_pool(name="xt", bufs=7))
        scr_pool = ctx.enter_context(tc.tile_pool(name="scr", bufs=4))
        sq_pool = ctx.enter_context(tc.tile_pool(name="sq", bufs=2))
        stat_pool = ctx.enter_context(tc.tile_pool(name="stat", bufs=8))
        qk_pool = ctx.enter_context(tc.tile_pool(name="qksb", bufs=10))
        qkn_pool = ctx.enter_context(tc.tile_pool(name="qkn", bufs=3))
        exp_pool = ctx.enter_context(tc.tile_pool(name="exp", bufs=6))
        acc_pool = ctx.enter_context(tc.tile_pool(name="acc", bufs=4))
        rro_pool = ctx.enter_context(tc.tile_pool(name="rro", bufs=2))
        yt_pool = ctx.enter_context(tc.tile_pool(name="yt", bufs=3))
        cpw_pool = ctx.enter_context(tc.tile_pool(name="cpw", bufs=16))
        yall_pool = ctx.enter_context(tc.tile_pool(name="yall", bufs=4))
        osb_pool = ctx.enter_context(tc.tile_pool(name="osb", bufs=2))
        ps = ctx.enter_context(tc.tile_pool(name="ps", bufs=5, space="PSUM"))
        psy = ctx.enter_context(tc.tile_pool(name="psy", bufs=2,
                                             space="PSUM"))
        aux = ctx.enter_context(tc.tile_pool(name="aux", bufs=1,
                                             space="PSUM"))
        dram = ctx.enter_context(tc.tile_pool(name="dram", bufs=1,
                                              space="DRAM"))

        # ---- weights first so their DMAs lead the queues; k=0 block
        # separately so the first matmul can start early, the rest in one
        # DMA (8 ring-paced issues kept the ACT queue busy ~12us) ----
        wqkv_sb = wqkv_pool.tile([P, D // P, 3 * HD], MMD)
        w_qkv_v = w_qkv.ap().rearrange("(k p) e -> p k e", p=P)
        nc.scalar.dma_start(out=wqkv_sb[:, 0, :], in_=w_qkv_v[:, 0, :])
        nc.scalar.dma_start(out=wqkv_sb[:, 1:, :], in_=w_qkv_v[:, 1:, :])
        lam_sb = const.tile([P, 2], F32)
        nc.scalar.dma_start(out=lam_sb[:], in_=lam.ap())
        # host-packed rope tables + value embeddings: one contiguous DMA
        # each on the gpsimd queue
        cos_sb = const.tile([P, NT, QUARTER], MMD, name="cos_sb")
        sin_sb = const.tile([P, NT, QUARTER], MMD, name="sin_sb")
        ve_sb = const.tile([P, NT, HD], MMD, name="ve_sb")
        nc.gpsimd.dma_start(out=ve_sb[:], in_=ve_h.ap())
        nc.gpsimd.dma_start(out=cos_sb[:], in_=cos_t.ap())
        nc.gpsimd.dma_start(out=sin_sb[:], in_=sin_t.ap())

        # ---- constants ----
        ones_f = const.tile([P, P], F32)
        nc.vector.memset(ones_f[:], 1.0)
        ones_mat = const.tile([P, P], MMD)
        nc.vector.tensor_copy(ones_mat[:], ones_f[:])
        expb_col = const.tile([P, 1], F32)
        nc.vector.memset(expb_col[:], EXP_BIAS)
        ident_f = const.tile([P, P], F32)
        make_identity(nc, ident_f)
        ident = const.tile([P, P], MMD)
        nc.vector.tensor_copy(ident[:], ident_f[:])
        # warm the ACT exp table during startup (off critical path)
        warm = const.tile([P, 1], MMD)
        nc.scalar.activation(warm[:], expb_col[:],
                             mybir.ActivationFunctionType.Exp,
                             bias=expb_col[:])

        # ---- persistent per-block tensors ----
        kT_t = [big.tile([P, P], MMD, name=f"kT{j}") for j in range(NT)]
        v_t = [big.tile([P, HD], MMD, name=f"v{j}") for j in range(NT)]
        qT_c = [big.tile([P, TCH], MMD, name=f"qT{c}") for c in range(NC_CH)]

        cc_in = [dram.tile([N_CORES * P * P], MMD, name=f"ccin{k}")
                 for k in range(NPAIR)]
        cc_out = [dram.tile([N_CORES * P * P], MMD, name=f"ccout{k}")
                  for k in range(NPAIR)]
        cc_in_v = [t[:].rearrange("(j p f) -> j p f", j=N_CORES, p=P)
                   for t in cc_in]
        cc_out_v = [t[:].rearrange("(j p f) -> j p f", j=N_CORES, p=P)
                    for t in cc_out]

        xt_tiles = {}

        def ensure_xt(i):  # i even: tile pair (i, i+1)
            if i in xt_tiles or i >= NT:
                return
            xt = xt_pool.tile([P, D // P, 2 * P], MMD, tag="xt",
                              name=f"xt{i}")
            nc.sync.dma_start(
                out=xt[:],
                in_=x_t.ap().rearrange("(k p) t -> p k t", p=P)
                    [:, :, i * P:(i + 2) * P])
            xt_tiles[i] = xt

        ssq_g, qkn_g = {}, {}
        qk_tiles = {}
        yall = {}

        def qkv_tile_unit(g, ii):
            """QKV matmuls + v-mix + q/k eviction + sum-of-squares for one
            128-row tile."""
            i = 4 * g + ii
            if ii == 0:
                ssq_g[g] = stat_pool.tile([P, 8], F32, tag="ssq",
                                          name=f"ssq{g}")
                ensure_xt(4 * g + 4)
                ensure_xt(4 * g + 6)
            xt_huge = xt_tiles[i - i % 2]
            xoff = (i % 2) * P
            ps_qkv = ps.tile([P, 3 * HD], F32, tag="ps", name=f"psqkv{i}")
            for k in range(D // P):
                nc.tensor.matmul(ps_qkv[:], xt_huge[:, k, xoff:xoff + P],
                                 wqkv_sb[:, k, :],
                                 start=(k == 0), stop=(k == D // P - 1))
            nc.vector.scalar_tensor_tensor(
                out=v_t[i][:], in0=ps_qkv[:, 2 * HD:3 * HD],
                scalar=lam_sb[:, 0:1], in1=ve_sb[:, i, :],
                op0=mybir.AluOpType.mult, op1=mybir.AluOpType.add)
            qk_sb = qk_pool.tile([P, 2 * HD], MMD, tag="qksb",
                                 name=f"qksb{i}")
            nc.scalar.copy(qk_sb[:], ps_qkv[:, 0:2 * HD])
            qk_tiles[i] = qk_sb
            for half in range(2):
                sq = sq_pool.tile([P, HD], F32, tag="sq")
                nc.vector.scalar_tensor_tensor(
                    out=sq[:], in0=qk_sb[:, half * HD:(half + 1) * HD],
                    scalar=1.0, in1=qk_sb[:, half * HD:(half + 1) * HD],
                    op0=mybir.AluOpType.mult, op1=mybir.AluOpType.mult,
                    accum_out=ssq_g[g][:, 2 * ii + half:2 * ii + half + 1])

        def norm_a_unit(g):
            """Batched rsqrt via DVE integer magic + 2 Newton steps, then
            q/k normalize+scale into the group's qkn tile (fp16)."""
            sg = ssq_g[g]
            h_i = stat_pool.tile([P, 8], I32, tag="h_i")
            nc.vector.tensor_scalar(
                out=h_i[:], in0=sg[:].bitcast(I32), scalar1=1,
                scalar2=None, op0=mybir.AluOpType.logical_shift_right)
            y0 = stat_pool.tile([P, 8], F32, tag="y0")
            nc.vector.tensor_scalar(
                out=y0[:].bitcast(I32), in0=h_i[:], scalar1=-1,
                scalar2=RSQRT_MAGIC,
                op0=mybir.AluOpType.mult, op1=mybir.AluOpType.add)
            t1 = stat_pool.tile([P, 8], F32, tag="t1")
            rsq = stat_pool.tile([P, 8], F32, tag="rsq", name=f"rsq{g}")
            cur = y0
            for it, nxt in ((0, t1), (1, rsq)):
                tt = stat_pool.tile([P, 8], F32, tag=f"tt{it}")
                nc.vector.tensor_mul(tt[:], cur[:], cur[:])
                nc.vector.tensor_mul(tt[:], tt[:], sg[:])
                nc.vector.tensor_scalar(
                    out=tt[:], in0=tt[:], scalar1=-0.5, scalar2=1.5,
                    op0=mybir.AluOpType.mult, op1=mybir.AluOpType.add)
                nc.vector.tensor_mul(nxt[:], cur[:], tt[:])
                cur = nxt
            sq128 = float(np.sqrt(HD))
            qkn = qkn_pool.tile([P, 4, 2 * HD], MMD, tag="qkn",
                                name=f"qkn{g}")
            qkn_g[g] = qkn
            for ii in range(4):
                qk_sb = qk_tiles[4 * g + ii]
                nc.vector.tensor_scalar(
                    out=qkn[:, ii, 0:HD], in0=qk_sb[:, 0:HD],
                    scalar1=rsq[:, 2 * ii:2 * ii + 1],
                    scalar2=ATTN_SCALE * sq128,
                    op0=mybir.AluOpType.mult, op1=mybir.AluOpType.mult)
                nc.vector.tensor_scalar(
                    out=qkn[:, ii, HD:2 * HD], in0=qk_sb[:, HD:2 * HD],
                    scalar1=rsq[:, 2 * ii + 1:2 * ii + 2], scalar2=sq128,
                    op0=mybir.AluOpType.mult, op1=mybir.AluOpType.mult)

        def norm_b_unit(g):
            """RoPE on the first-quarter pairs of q AND k of all 4 tiles in
            6 batched DVE ops ([P, 4, 2, 32] access patterns)."""
            qkn = qkn_g[g]

            def rope_rng(col0):
                s = qkn[:]
                return bass.AP(s.tensor, s.offset + col0,
                               [list(s.ap[0]), [2 * HD, 4], [HD, 2],
                                [1, QUARTER]])

            def cs_rng(tl):
                s = tl[:]
                return bass.AP(s.tensor, s.offset + 4 * g * QUARTER,
                               [list(s.ap[0]), [QUARTER, 4], [0, 2],
                                [1, QUARTER]])

            x1 = rope_rng(0)
            x2 = rope_rng(2 * QUARTER)
            cb_, sb_ = cs_rng(cos_sb), cs_rng(sin_sb)
            a = scr_pool.tile([P, 4, 2, QUARTER], MMD, tag="ropeA")
            b = scr_pool.tile([P, 4, 2, QUARTER], MMD, tag="ropeB")
            c2 = scr_pool.tile([P, 4, 2, QUARTER], MMD, tag="ropeC")
            d2 = scr_pool.tile([P, 4, 2, QUARTER], MMD, tag="ropeD")
            nc.vector.tensor_mul(a[:], x1, cb_)
            nc.vector.tensor_mul(b[:], x2, sb_)
            nc.vector.tensor_mul(c2[:], x2, cb_)
            nc.vector.tensor_mul(d2[:], x1, sb_)
            nc.vector.tensor_add(x1, a[:], b[:])
            nc.vector.tensor_sub(x2, c2[:], d2[:])

        def norm_c_unit(g):
            """PE transposes into [e, t] layout; DVE evicts PSUM. (XBAR
            dma_start_transpose was tried here and is 1.26us/tile — it
            congests the sync DMA queue and starves the x-tile loads.)"""
            qkn = qkn_g[g]
            for ii in range(4):
                i = 4 * g + ii
                for src, dst, c0 in ((qkn[:, ii, 0:HD], qT_c[g], ii * P),
                                     (qkn[:, ii, HD:2 * HD], kT_t[i], 0)):
                    # aux bank: keeps the hot "ps" pool for s_mm/qkv
                    ps_tr = aux.tile([P, P], MMD, tag="aux")
                    nc.tensor.transpose(ps_tr[:], src, ident[:])
                    nc.vector.tensor_copy(dst[:, c0:c0 + P], ps_tr[:])

        def recv_unit(k):
            ya = yall_pool.tile([P, N_CORES, P], MMD, tag="yall",
                                name=f"yall{k}")
            for h in range(N_CORES):
                nc.sync.dma_start(out=ya[:, h, :], in_=cc_out_v[k][h])
            yall[k] = ya

        def proj_unit(k, dh, pool=None):
            """Output projection for this core's row-block k, output
            columns [dh*512, dh*512+512)."""
            ps_o = (pool or aux).tile([P, TCH], F32,
                                      tag="aux" if pool is None else "psy",
                                      name=f"pso{k}_{dh}")
            for h in range(H):
                nc.tensor.matmul(ps_o[:], yall[k][:, h, :],
                                 cpw_tiles[(h, dh)][:],
                                 start=(h == 0), stop=(h == H - 1))
            o_sb = osb_pool.tile([P, TCH], MMD, tag="osb")
            nc.vector.tensor_copy(o_sb[:], ps_o[:])
            nc.sync.dma_start(
                out=y_shard.ap()[k * P:(k + 1) * P,
                                 dh * TCH:(dh + 1) * TCH],
                in_=o_sb[:])

        # ------------------------------------------------------------------
        def attn_pair(k, fillers):
            """Interleaved attention for chunks (2k, 2k+1); pops one filler
            unit after each attention step. Fires AllToAll k at the end."""
            cA, cB = 2 * k, 2 * k + 1
            jA, jB = 4 * cA + 4, 4 * cB + 4
            psys, accs, s_pre, e_pre = {}, {}, {}, {}
            n_fill = max(1, len(fillers))
            n_steps = jA + jB
            state = {"step": 0, "done": 0}

            def s_mm(c, j):
                # diagonal block j = 4c+m: only query columns >= m*128 of
                # the chunk can attend to this key block
                m = max(0, j - 4 * c)
                w = TCH - m * P
                p_s = ps.tile([P, TCH], F32, tag="ps", name=f"s{c}_{j}")
                nc.tensor.matmul(p_s[:, 0:w], kT_t[j][:],
                                 qT_c[c][:, m * P:TCH],
                                 start=True, stop=True)
                return p_s

            def filler():
                # pace fillers evenly over the pair's attention steps so
                # late units (projection of the previous collective) are
                # consumed late, when that collective has surely landed
                state["step"] += 1
                while (fillers and
                       state["done"] < state["step"] * n_fill // n_steps):
                    state["done"] += 1
                    fillers.pop(0)()

            def exp_mask_add(c, j):
                # exp/mask/denominator-accumulate for block j, one step
                # ahead of its AV matmul so the PE never waits on ACT
                p_s = s_pre.pop((c, j))
                m = max(0, j - 4 * c)
                w = TCH - m * P
                e = exp_pool.tile([P, TCH], MMD, tag="e")
                nc.scalar.activation(e[:, 0:w], p_s[:, 0:w],
                                     mybir.ActivationFunctionType.Exp,
                                     bias=expb_col[:])
                if j >= 4 * c:
                    # triangular mask on the first 128 columns only
                    nc.gpsimd.affine_select(
                        out=e[:, 0:P], in_=e[:, 0:P],
                        compare_op=mybir.AluOpType.is_ge, fill=0.0,
                        base=0, channel_multiplier=-1,
                        pattern=[[1, P]])
                if j == 0:
                    # separate acc tile: the j+1 add runs ahead of y_mm(0),
                    # so e0 itself must stay unmodified for the AV matmul
                    acc = acc_pool.tile([P, TCH], MMD, tag="acc",
                                        name=f"acc{c}")
                    nc.vector.tensor_copy(acc[:], e[:])
                    accs[c] = acc
                else:
                    nc.vector.tensor_add(accs[c][:, m * P:TCH],
                                         accs[c][:, m * P:TCH],
                                         e[:, 0:w])
                e_pre[(c, j)] = e

            def prime(c):
                s_pre[(c, 0)] = s_mm(c, 0)
                s_pre[(c, 1)] = s_mm(c, 1)
                exp_mask_add(c, 0)

            def step(c, j, jmax):
                if j + 2 < jmax:
                    s_pre[(c, j + 2)] = s_mm(c, j + 2)
                if j + 1 < jmax:
                    exp_mask_add(c, j + 1)
                if j == 0:
                    # lazy: waits only for the previous pair's flush of the
                    # matching half, not both
                    psys[c] = psy.tile([P, TCH], F32, tag="psy",
                                       name=f"psy{c}")
                m = max(0, j - 4 * c)
                e = e_pre.pop((c, j))
                nc.tensor.matmul(psys[c][:, m * P:TCH], v_t[j][:],
                                 e[:, 0:TCH - m * P], start=(j == 0),
                                 stop=(j == jmax - 1),
                                 skip_group_check=True)

            def flush(c, half):
                ps_rb = aux.tile([P, TCH], F32, tag="aux", name=f"rb{c}")
                nc.tensor.matmul(ps_rb[:], ones_mat[:], accs[c][:],
                                 start=True, stop=True)
                rb = rro_pool.tile([P, TCH], F32, tag="rro")
                nc.vector.reciprocal_approx_fast(out=rb[:], in_=ps_rb[:])
                yT = yt_pool.tile([P, TCH], MMD, tag="yt", name=f"yT{c}")
                nc.vector.tensor_mul(yT[:], psys[c][:], rb[:])
                for m in range(4):
                    nc.gpsimd.dma_start(out=cc_in_v[k][4 * half + m],
                                        in_=yT[:, m * P:(m + 1) * P])

            prime(cA)
            prime(cB)
            for j in range(jB):
                if j < jA:
                    step(cA, j, jA)
                    filler()
                if j == jA:
                    flush(cA, 0)
                step(cB, j, jB)
                filler()
            flush(cB, 1)

            nc.gpsimd.collective_compute(
                "AllToAll", mybir.AluOpType.bypass,
                replica_groups=[list(range(N_CORES))],
                ins=[cc_in[k][:].opt()], outs=[cc_out[k][:].opt()])
            # leftover fillers (filler list should normally be empty here)
            while fillers:
                fillers.pop(0)()

        # ------------------------------------------------------------------
        # prologue: groups 0, 1 inline
        ensure_xt(0)
        ensure_xt(2)
        for g in (0, 1):
            for ii in range(4):
                qkv_tile_unit(g, ii)
        for g in (0, 1):
            norm_a_unit(g)
            norm_b_unit(g)
            norm_c_unit(g)

        def load_cpw():
            # output-projection weights on the sync queue; emitted at pair 1
            # start so they don't delay pair 0/1's x-tile loads (needed from
            # proj_unit(0), a pair-1 filler)
            for h in range(H):
                for dh in range(D // TCH):
                    ct = cpw_pool.tile([P, TCH], MMD, tag="cpw",
                                       name=f"cpw{h}_{dh}")
                    nc.sync.dma_start(
                        out=ct[:],
                        in_=cpw.ap()[h * P:(h + 1) * P,
                                     dh * TCH:(dh + 1) * TCH])
                    cpw_tiles[(h, dh)] = ct

        cpw_tiles = {}

        for k in range(NPAIR):
            if k == 1:
                load_cpw()
            gA, gB = 2 * k + 2, 2 * k + 3
            fillers = []
            # receive first (cheap; soft-syncs cores once per pair), then
            # per group qkv tiles + DVE norm chain (rsqrt+scales, rope); PE
            # transposes after every rope; projection consumed last so the
            # previous collective has a full pair of slack
            if k >= 1:
                fillers.append(lambda k=k - 1: recv_unit(k))
            for g in (gA, gB):
                if g < NC_CH:
                    for ii in range(4):
                        fillers.append(
                            lambda g=g, ii=ii: qkv_tile_unit(g, ii))
                    fillers.append(lambda g=g: norm_a_unit(g))
                    fillers.append(lambda g=g: norm_b_unit(g))
            for g in (gA, gB):
                if g < NC_CH:
                    fillers.append(lambda g=g: norm_c_unit(g))
            if k >= 1:
                for dh in range(D // TCH):
                    fillers.append(
                        lambda k=k - 1, dh=dh: proj_unit(k, dh))
            attn_pair(k, fillers)

        # epilogue: last pair's projection (on the freed psy banks so the
        # two output halves don't serialize on the single aux bank)
        recv_unit(NPAIR - 1)
        for dh in range(D // TCH):
            proj_unit(NPAIR - 1, dh, pool=psy)

    nc.compile()
    return nc


def _host_prep(x, ve, qkv_w, lambdas, c_proj_w):
    x = np.asarray(x, dtype=np.float32)
    ve = np.asarray(ve, dtype=np.float32)
    qkv_w = np.asarray(qkv_w, dtype=np.float32)
    lambdas = np.asarray(lambdas, dtype=np.float32)
    c_proj_w = np.asarray(c_proj_w, dtype=np.float32)

    xT = np.ascontiguousarray(x[0].T.astype(NP_MMD))
    cpwT = np.ascontiguousarray(c_proj_w.T.astype(NP_MMD))
    lam_b = np.ascontiguousarray(np.broadcast_to(lambdas, (P, 2)))

    angular = (np.float32(1.0 / 1024.0)
               ** np.linspace(0.0, 1.0, QUARTER, dtype=np.float32))
    t = np.arange(T, dtype=np.float32)
    theta = t[:, None] * angular[None, :]

    def pack(a):  # [T, F] -> [P, NT*F] with row t = tile*128 + p
        F = a.shape[1]
        return np.ascontiguousarray(
            a.reshape(NT, P, F).transpose(1, 0, 2).reshape(P, NT * F)
            .astype(NP_MMD))

    cos_pk = pack(np.cos(theta))
    sin_pk = pack(np.sin(theta))

    in_maps = []
    for h in range(N_CORES):
        sl = slice(h * HD, (h + 1) * HD)
        w_qkvT = np.ascontiguousarray(np.concatenate(
            [qkv_w[0, sl, :].T, qkv_w[1, sl, :].T, qkv_w[2, sl, :].T],
            axis=1).astype(NP_MMD))
        in_maps.append({
            "x_t": xT,
            "w_qkv": w_qkvT,
            "cos_t": cos_pk,
            "sin_t": sin_pk,
            "ve_h": pack(ve[0][:, sl] * lambdas[1]),
            "lam": lam_b,
            "cpw": cpwT,
        })
    return in_maps


def kernel(x, ve, qkv_w, lambdas, c_proj_w, _trace=False, _trace_kwargs=None):
    if "nc" not in _cached:
        _cached["nc"] = build_module()
    nc = _cached["nc"]
    in_maps = _host_prep(x, ve, qkv_w, lambdas, c_proj_w)
    kw = {}
    if _trace:
        kw = dict(trace=True, **(_trace_kwargs or {}))
    res = run_bass_kernel_spmd(nc, in_maps, core_ids=list(range(N_CORES)),
                               **kw)
    _cached["last_result"] = res
    blocks = np.empty((NT, P, D), dtype=np.float32)
    for j in range(N_CORES):
        sh = np.asarray(res.results[j]["y_shard"],
                        dtype=np.float32).reshape(4, P, D)
        for k in range(4):
            blocks[8 * k + j] = sh[k]
    return blocks.reshape(1, T, D)


# revision 38
# speedup vs baseline: 1.0597x; 1.0597x over previous
"""Trainium2 Bass kernel: causal self-attention (modded-nanogpt style),
tensor-parallel over heads across 8 NeuronCores with PHASED AllToAll
re-shards overlapped with attention compute.

Self-contained: hardcodes B=1, T=4096, D=1024, H=8, Hd=128, scale=0.12.

Per-core program (core = head). Query chunks of 512 rows are processed in
PAIRS (0,1)(2,3)(4,5)(6,7); within a pair the two chunks' S/AV matmuls are
interleaved so the PE pipeline never drains (TRN2 PE p-state ramps to full
clock only after ~3us of continuous execution). qkv-projection, q/k norm +
rope, and output-projection work for other chunks is emitted as FILLER
between attention steps to absorb exp-latency bubbles.

After each pair, that pair's 8 query blocks (128 rows each) are re-sharded
head->sequence with a small AllToAll (block 8k+j -> core j, slot layout
[8, 128, 128] fp16 = 256KB); 3 of the 4 collectives plus 3/4 of the output
projection are fully hidden under attention compute of later pairs.

Softmax denominator: exp tiles are accumulated on DVE (fp16), reduced over
the key axis with an all-ones [128,128] matmul (which also broadcasts the
row across all PSUM partitions), inverted with a single custom-DVE
reciprocal_approx_fast op, and multiplied into y^T.

Engine assignment: ACT = exp only; DVE = softmax accumulation, q/k scales,
batched rope, rsqrt/reciprocal magic; Pool/GpSimd = causal masks
(affine_select in place), PSUM evictions, sum-of-squares, v-mix, small
DMAs; PE = all matmuls; Sync = big DMAs.
"""

import os
import sys

sys.path.insert(0, "/opt/trn_rl_repo")

from contextlib import ExitStack

import numpy as np

import concourse.bass as bass
import concourse.bacc as bacc
import concourse.mybir as mybir
import concourse.tile as tile
from concourse.bass_utils import run_bass_kernel_spmd
from concourse.masks import make_identity

N_CORES = 8
T = 4096
D = 1024
H = 8
HD = 128
ATTN_SCALE = 0.12
P = 128
TCH = 512
NT = T // P          # 32 t-tiles (query/key blocks of 128)
NC_CH = T // TCH     # 8 chunks
NPAIR = NC_CH // 2   # 4 chunk pairs == 4 collectives
QUARTER = HD // 4

F32 = mybir.dt.float32
I32 = mybir.dt.int32
MMD = mybir.dt.float16
NP_MMD = np.float16
# exp(s - 12*ln2) = 2^-12 * exp(s): keeps fp16 exp values and their fp16
# partial sums in range; the scaling cancels in the softmax normalize.
EXP_BIAS = -8.317766166719343
RSQRT_MAGIC = 0x5F3759DF

_cached = {}


def build_module():
    nc = bacc.Bacc("TRN2", target_bir_lowering=False, debug=False,
                   num_devices=N_CORES)

    x_t = nc.dram_tensor("x_t", [D, T], MMD, kind="ExternalInput")
    w_qkv = nc.dram_tensor("w_qkv", [D, 3 * HD], MMD, kind="ExternalInput")
    # host-packed [p, tile, freq]/[p, tile, e] layouts -> one contiguous
    # DMA each (the natural [T, .] layouts DMA at 64B-element granularity)
    cos_t = nc.dram_tensor("cos_t", [P, NT * QUARTER], MMD,
                           kind="ExternalInput")
    sin_t = nc.dram_tensor("sin_t", [P, NT * QUARTER], MMD,
                           kind="ExternalInput")
    ve_h = nc.dram_tensor("ve_h", [P, NT * HD], MMD, kind="ExternalInput")
    lam = nc.dram_tensor("lam", [P, 2], F32, kind="ExternalInput")
    cpw = nc.dram_tensor("cpw", [D, D], MMD, kind="ExternalInput")
    # 4 row-blocks of 128: block (8k + core) lands at position k
    y_shard = nc.dram_tensor("y_shard", [4 * P, D], MMD,
                             kind="ExternalOutput")

    with tile.TileContext(nc) as tc, nc.allow_low_precision(
            reason="reduced-precision matmul operands"), ExitStack() as ctx:
        const = ctx.enter_context(tc.tile_pool(name="const", bufs=1))
        wqkv_pool = ctx.enter_context(tc.tile_pool(name="wqkv", bufs=1))
        big = ctx.enter_context(tc.tile_pool(name="big", bufs=1))
        xt_pool = ctx.enter_context(tc.tile_pool(name="xt", bufs=7))
        scr_pool = ctx.enter_context(tc.tile_pool(name="scr", bufs=4))
        sq_pool = ctx.enter_context(tc.tile_pool(name="sq", bufs=2))
        stat_pool = ctx.enter_context(tc.tile_pool(name="stat", bufs=8))
        qk_pool = ctx.enter_context(tc.tile_pool(name="qksb", bufs=10))
        qkn_pool = ctx.enter_context(tc.tile_pool(name="qkn", bufs=3))
        exp_pool = ctx.enter_context(tc.tile_pool(name="exp", bufs=6))
        acc_pool = ctx.enter_context(tc.tile_pool(name="acc", bufs=4))
        rro_pool = ctx.enter_context(tc.tile_pool(name="rro", bufs=2))
        yt_pool = ctx.enter_context(tc.tile_pool(name="yt", bufs=3))
        cpw_pool = ctx.enter_context(tc.tile_pool(name="cpw", bufs=16))
        yall_pool = ctx.enter_context(tc.tile_pool(name="yall", bufs=4))
        osb_pool = ctx.enter_context(tc.tile_pool(name="osb", bufs=2))
        ps = ctx.enter_context(tc.tile_pool(name="ps", bufs=5, space="PSUM"))
        psy = ctx.enter_context(tc.tile_pool(name="psy", bufs=2,
                                             space="PSUM"))
        aux = ctx.enter_context(tc.tile_pool(name="aux", bufs=1,
                                             space="PSUM"))
        dram = ctx.enter_context(tc.tile_pool(name="dram", bufs=1,
                                              space="DRAM"))

        # ---- weights first so their DMAs lead the queues; k=0 block
        # separately so the first matmul can start early, the rest in one
        # DMA (8 ring-paced issues kept the ACT queue busy ~12us) ----
        wqkv_sb = wqkv_pool.tile([P, D // P, 3 * HD], MMD)
        w_qkv_v = w_qkv.ap().rearrange("(k p) e -> p k e", p=P)
        nc.scalar.dma_start(out=wqkv_sb[:, 0, :], in_=w_qkv_v[:, 0, :])
        nc.scalar.dma_start(out=wqkv_sb[:, 1:, :], in_=w_qkv_v[:, 1:, :])
        lam_sb = const.tile([P, 2], F32)
        nc.scalar.dma_start(out=lam_sb[:], in_=lam.ap())
        # host-packed rope tables + value embeddings: one contiguous DMA
        # each on the gpsimd queue
        cos_sb = const.tile([P, NT, QUARTER], MMD, name="cos_sb")
        sin_sb = const.tile([P, NT, QUARTER], MMD, name="sin_sb")
        ve_sb = const.tile([P, NT, HD], MMD, name="ve_sb")
        ve_h_v = ve_h.ap().rearrange("p (n e) -> p n e", e=HD)
        nc.gpsimd.dma_start(out=ve_sb[:, 0:8, :], in_=ve_h_v[:, 0:8, :])
        nc.gpsimd.dma_start(out=cos_sb[:], in_=cos_t.ap())
        nc.gpsimd.dma_start(out=sin_sb[:], in_=sin_t.ap())
        nc.gpsimd.dma_start(out=ve_sb[:, 8:, :], in_=ve_h_v[:, 8:, :])

        # ---- constants ----
        ones_f = const.tile([P, P], F32)
        nc.vector.memset(ones_f[:], 1.0)
        ones_mat = const.tile([P, P], MMD)
        nc.vector.tensor_copy(ones_mat[:], ones_f[:])
        expb_col = const.tile([P, 1], F32)
        nc.vector.memset(expb_col[:], EXP_BIAS)
        ident_f = const.tile([P, P], F32)
        make_identity(nc, ident_f)
        ident = const.tile([P, P], MMD)
        nc.vector.tensor_copy(ident[:], ident_f[:])
        # warm the ACT exp table during startup (off critical path)
        warm = const.tile([P, 1], MMD)
        nc.scalar.activation(warm[:], expb_col[:],
                             mybir.ActivationFunctionType.Exp,
                             bias=expb_col[:])

        # ---- persistent per-block tensors ----
        kT_t = [big.tile([P, P], MMD, name=f"kT{j}") for j in range(NT)]
        v_t = [big.tile([P, HD], MMD, name=f"v{j}") for j in range(NT)]
        qT_c = [big.tile([P, TCH], MMD, name=f"qT{c}") for c in range(NC_CH)]

        cc_in = [dram.tile([N_CORES * P * P], MMD, name=f"ccin{k}")
                 for k in range(NPAIR)]
        cc_out = [dram.tile([N_CORES * P * P], MMD, name=f"ccout{k}")
                  for k in range(NPAIR)]
        cc_in_v = [t[:].rearrange("(j p f) -> j p f", j=N_CORES, p=P)
                   for t in cc_in]
        cc_out_v = [t[:].rearrange("(j p f) -> j p f", j=N_CORES, p=P)
                    for t in cc_out]

        xt_tiles = {}

        def ensure_xt(i):  # i even: tile pair (i, i+1)
            if i in xt_tiles or i >= NT:
                return
            xt = xt_pool.tile([P, D // P, 2 * P], MMD, tag="xt",
                              name=f"xt{i}")
            nc.sync.dma_start(
                out=xt[:],
                in_=x_t.ap().rearrange("(k p) t -> p k t", p=P)
                    [:, :, i * P:(i + 2) * P])
            xt_tiles[i] = xt

        ssq_g, qkn_g = {}, {}
        qk_tiles = {}
        yall = {}

        def qkv_tile_unit(g, ii):
            """QKV matmuls + v-mix + q/k eviction + sum-of-squares for one
            128-row tile."""
            i = 4 * g + ii
            if ii == 0:
                ssq_g[g] = stat_pool.tile([P, 8], F32, tag="ssq",
                                          name=f"ssq{g}")
                ensure_xt(4 * g + 4)
                ensure_xt(4 * g + 6)
            xt_huge = xt_tiles[i - i % 2]
            xoff = (i % 2) * P
            ps_qkv = ps.tile([P, 3 * HD], F32, tag="ps", name=f"psqkv{i}")
            for k in range(D // P):
                nc.tensor.matmul(ps_qkv[:], xt_huge[:, k, xoff:xoff + P],
                                 wqkv_sb[:, k, :],
                                 start=(k == 0), stop=(k == D // P - 1))
            nc.vector.scalar_tensor_tensor(
                out=v_t[i][:], in0=ps_qkv[:, 2 * HD:3 * HD],
                scalar=lam_sb[:, 0:1], in1=ve_sb[:, i, :],
                op0=mybir.AluOpType.mult, op1=mybir.AluOpType.add)
            qk_sb = qk_pool.tile([P, 2 * HD], MMD, tag="qksb",
                                 name=f"qksb{i}")
            nc.scalar.copy(qk_sb[:], ps_qkv[:, 0:2 * HD])
            qk_tiles[i] = qk_sb
            for half in range(2):
                sq = sq_pool.tile([P, HD], F32, tag="sq")
                nc.vector.scalar_tensor_tensor(
                    out=sq[:], in0=qk_sb[:, half * HD:(half + 1) * HD],
                    scalar=1.0, in1=qk_sb[:, half * HD:(half + 1) * HD],
                    op0=mybir.AluOpType.mult, op1=mybir.AluOpType.mult,
                    accum_out=ssq_g[g][:, 2 * ii + half:2 * ii + half + 1])

        def norm_a_unit(g):
            """Batched rsqrt via DVE integer magic + 2 Newton steps, then
            q/k normalize+scale into the group's qkn tile (fp16)."""
            sg = ssq_g[g]
            h_i = stat_pool.tile([P, 8], I32, tag="h_i")
            nc.vector.tensor_scalar(
                out=h_i[:], in0=sg[:].bitcast(I32), scalar1=1,
                scalar2=None, op0=mybir.AluOpType.logical_shift_right)
            y0 = stat_pool.tile([P, 8], F32, tag="y0")
            nc.vector.tensor_scalar(
                out=y0[:].bitcast(I32), in0=h_i[:], scalar1=-1,
                scalar2=RSQRT_MAGIC,
                op0=mybir.AluOpType.mult, op1=mybir.AluOpType.add)
            t1 = stat_pool.tile([P, 8], F32, tag="t1")
            rsq = stat_pool.tile([P, 8], F32, tag="rsq", name=f"rsq{g}")
            cur = y0
            for it, nxt in ((0, t1), (1, rsq)):
                tt = stat_pool.tile([P, 8], F32, tag=f"tt{it}")
                nc.vector.tensor_mul(tt[:], cur[:], cur[:])
                nc.vector.tensor_mul(tt[:], tt[:], sg[:])
                nc.vector.tensor_scalar(
                    out=tt[:], in0=tt[:], scalar1=-0.5, scalar2=1.5,
                    op0=mybir.AluOpType.mult, op1=mybir.AluOpType.add)
                nc.vector.tensor_mul(nxt[:], cur[:], tt[:])
                cur = nxt
            sq128 = float(np.sqrt(HD))
            qkn = qkn_pool.tile([P, 4, 2 * HD], MMD, tag="qkn",
                                name=f"qkn{g}")
            qkn_g[g] = qkn
            for ii in range(4):
                qk_sb = qk_tiles[4 * g + ii]
                nc.vector.tensor_scalar(
                    out=qkn[:, ii, 0:HD], in0=qk_sb[:, 0:HD],
                    scalar1=rsq[:, 2 * ii:2 * ii + 1],
                    scalar2=ATTN_SCALE * sq128,
                    op0=mybir.AluOpType.mult, op1=mybir.AluOpType.mult)
                nc.vector.tensor_scalar(
                    out=qkn[:, ii, HD:2 * HD], in0=qk_sb[:, HD:2 * HD],
                    scalar1=rsq[:, 2 * ii + 1:2 * ii + 2], scalar2=sq128,
                    op0=mybir.AluOpType.mult, op1=mybir.AluOpType.mult)

        def norm_b_unit(g):
            """RoPE on the first-quarter pairs of q AND k of all 4 tiles in
            6 batched DVE ops ([P, 4, 2, 32] access patterns)."""
            qkn = qkn_g[g]

            def rope_rng(col0):
                s = qkn[:]
                return bass.AP(s.tensor, s.offset + col0,
                               [list(s.ap[0]), [2 * HD, 4], [HD, 2],
                                [1, QUARTER]])

            def cs_rng(tl):
                s = tl[:]
                return bass.AP(s.tensor, s.offset + 4 * g * QUARTER,
                               [list(s.ap[0]), [QUARTER, 4], [0, 2],
                                [1, QUARTER]])

            x1 = rope_rng(0)
            x2 = rope_rng(2 * QUARTER)
            cb_, sb_ = cs_rng(cos_sb), cs_rng(sin_sb)
            a = scr_pool.tile([P, 4, 2, QUARTER], MMD, tag="ropeA")
            b = scr_pool.tile([P, 4, 2, QUARTER], MMD, tag="ropeB")
            c2 = scr_pool.tile([P, 4, 2, QUARTER], MMD, tag="ropeC")
            d2 = scr_pool.tile([P, 4, 2, QUARTER], MMD, tag="ropeD")
            nc.vector.tensor_mul(a[:], x1, cb_)
            nc.vector.tensor_mul(b[:], x2, sb_)
            nc.vector.tensor_mul(c2[:], x2, cb_)
            nc.vector.tensor_mul(d2[:], x1, sb_)
            nc.vector.tensor_add(x1, a[:], b[:])
            nc.vector.tensor_sub(x2, c2[:], d2[:])

        def norm_c_unit(g):
            """PE transposes into [e, t] layout; DVE evicts PSUM. (XBAR
            dma_start_transpose was tried here and is 1.26us/tile — it
            congests the sync DMA queue and starves the x-tile loads.)"""
            qkn = qkn_g[g]
            for ii in range(4):
                i = 4 * g + ii
                for src, dst, c0 in ((qkn[:, ii, 0:HD], qT_c[g], ii * P),
                                     (qkn[:, ii, HD:2 * HD], kT_t[i], 0)):
                    # aux bank: keeps the hot "ps" pool for s_mm/qkv
                    ps_tr = aux.tile([P, P], MMD, tag="aux")
                    nc.tensor.transpose(ps_tr[:], src, ident[:])
                    nc.vector.tensor_copy(dst[:, c0:c0 + P], ps_tr[:])

        def recv_unit(k):
            ya = yall_pool.tile([P, N_CORES, P], MMD, tag="yall",
                                name=f"yall{k}")
            for h in range(N_CORES):
                nc.sync.dma_start(out=ya[:, h, :], in_=cc_out_v[k][h])
            yall[k] = ya

        def proj_unit(k, dh, pool=None):
            """Output projection for this core's row-block k, output
            columns [dh*512, dh*512+512)."""
            ps_o = (pool or aux).tile([P, TCH], F32,
                                      tag="aux" if pool is None else "psy",
                                      name=f"pso{k}_{dh}")
            for h in range(H):
                nc.tensor.matmul(ps_o[:], yall[k][:, h, :],
                                 cpw_tiles[(h, dh)][:],
                                 start=(h == 0), stop=(h == H - 1))
            o_sb = osb_pool.tile([P, TCH], MMD, tag="osb")
            nc.vector.tensor_copy(o_sb[:], ps_o[:])
            nc.sync.dma_start(
                out=y_shard.ap()[k * P:(k + 1) * P,
                                 dh * TCH:(dh + 1) * TCH],
                in_=o_sb[:])

        # ------------------------------------------------------------------
        def attn_pair(k, fillers):
            """Interleaved attention for chunks (2k, 2k+1); pops one filler
            unit after each attention step. Fires AllToAll k at the end."""
            cA, cB = 2 * k, 2 * k + 1
            jA, jB = 4 * cA + 4, 4 * cB + 4
            psys, accs, s_pre, e_pre = {}, {}, {}, {}
            n_fill = max(1, len(fillers))
            n_steps = jA + jB
            state = {"step": 0, "done": 0}

            def s_mm(c, j):
                # diagonal block j = 4c+m: only query columns >= m*128 of
                # the chunk can attend to this key block
                m = max(0, j - 4 * c)
                w = TCH - m * P
                p_s = ps.tile([P, TCH], F32, tag="ps", name=f"s{c}_{j}")
                nc.tensor.matmul(p_s[:, 0:w], kT_t[j][:],
                                 qT_c[c][:, m * P:TCH],
                                 start=True, stop=True)
                return p_s

            def filler():
                # pace fillers evenly over the pair's attention steps so
                # late units (projection of the previous collective) are
                # consumed late, when that collective has surely landed
                state["step"] += 1
                while (fillers and
                       state["done"] < state["step"] * n_fill // n_steps):
                    state["done"] += 1
                    fillers.pop(0)()

            def exp_mask_add(c, j):
                # exp/mask/denominator-accumulate for block j, one step
                # ahead of its AV matmul so the PE never waits on ACT
                p_s = s_pre.pop((c, j))
                m = max(0, j - 4 * c)
                w = TCH - m * P
                e = exp_pool.tile([P, TCH], MMD, tag="e")
                nc.scalar.activation(e[:, 0:w], p_s[:, 0:w],
                                     mybir.ActivationFunctionType.Exp,
                                     bias=expb_col[:])
                if j >= 4 * c:
                    # triangular mask on the first 128 columns only
                    nc.gpsimd.affine_select(
                        out=e[:, 0:P], in_=e[:, 0:P],
                        compare_op=mybir.AluOpType.is_ge, fill=0.0,
                        base=0, channel_multiplier=-1,
                        pattern=[[1, P]])
                if j == 0:
                    # separate acc tile: the j+1 add runs ahead of y_mm(0),
                    # so e0 itself must stay unmodified for the AV matmul
                    acc = acc_pool.tile([P, TCH], MMD, tag="acc",
                                        name=f"acc{c}")
                    nc.vector.tensor_copy(acc[:], e[:])
                    accs[c] = acc
                else:
                    nc.vector.tensor_add(accs[c][:, m * P:TCH],
                                         accs[c][:, m * P:TCH],
                                         e[:, 0:w])
                e_pre[(c, j)] = e

            def prime(c):
                s_pre[(c, 0)] = s_mm(c, 0)
                s_pre[(c, 1)] = s_mm(c, 1)
                exp_mask_add(c, 0)

            def step(c, j, jmax):
                if j + 2 < jmax:
                    s_pre[(c, j + 2)] = s_mm(c, j + 2)
                if j + 1 < jmax:
                    exp_mask_add(c, j + 1)
                if j == 0:
                    # lazy: waits only for the previous pair's flush of the
                    # matching half, not both
                    psys[c] = psy.tile([P, TCH], F32, tag="psy",
                                       name=f"psy{c}")
                m = max(0, j - 4 * c)
                e = e_pre.pop((c, j))
                nc.tensor.matmul(psys[c][:, m * P:TCH], v_t[j][:],
                                 e[:, 0:TCH - m * P], start=(j == 0),
                                 stop=(j == jmax - 1),
                                 skip_group_check=True)

            def flush(c, half):
                ps_rb = aux.tile([P, TCH], F32, tag="aux", name=f"rb{c}")
                nc.tensor.matmul(ps_rb[:], ones_mat[:], accs[c][:],
                                 start=True, stop=True)
                rb = rro_pool.tile([P, TCH], F32, tag="rro")
                nc.vector.reciprocal_approx_fast(out=rb[:], in_=ps_rb[:])
                yT = yt_pool.tile([P, TCH], MMD, tag="yt", name=f"yT{c}")
                nc.vector.tensor_mul(yT[:], psys[c][:], rb[:])
                for m in range(4):
                    nc.gpsimd.dma_start(out=cc_in_v[k][4 * half + m],
                                        in_=yT[:, m * P:(m + 1) * P])

            prime(cA)
            prime(cB)
            for j in range(jB):
                if j < jA:
                    step(cA, j, jA)
                    filler()
                if j == jA:
                    flush(cA, 0)
                step(cB, j, jB)
                filler()
            flush(cB, 1)

            nc.gpsimd.collective_compute(
                "AllToAll", mybir.AluOpType.bypass,
                replica_groups=[list(range(N_CORES))],
                ins=[cc_in[k][:].opt()], outs=[cc_out[k][:].opt()])
            # leftover fillers (filler list should normally be empty here)
            while fillers:
                fillers.pop(0)()

        # ------------------------------------------------------------------
        # prologue: groups 0, 1 inline
        ensure_xt(0)
        ensure_xt(2)
        for g in (0, 1):
            for ii in range(4):
                qkv_tile_unit(g, ii)
        for g in (0, 1):
            norm_a_unit(g)
            norm_b_unit(g)
            norm_c_unit(g)

        def load_cpw():
            # output-projection weights on the sync queue; emitted at pair 1
            # start so they don't delay pair 0/1's x-tile loads (needed from
            # proj_unit(0), a pair-1 filler)
            for h in range(H):
                for dh in range(D // TCH):
                    ct = cpw_pool.tile([P, TCH], MMD, tag="cpw",
                                       name=f"cpw{h}_{dh}")
                    nc.sync.dma_start(
                        out=ct[:],
                        in_=cpw.ap()[h * P:(h + 1) * P,
                                     dh * TCH:(dh + 1) * TCH])
                    cpw_tiles[(h, dh)] = ct

        cpw_tiles = {}

        for k in range(NPAIR):
            if k == 1:
                load_cpw()
            gA, gB = 2 * k + 2, 2 * k + 3
            fillers = []
            # receive first (cheap; soft-syncs cores once per pair), then
            # per group qkv tiles + DVE norm chain (rsqrt+scales, rope); PE
            # transposes after every rope; projection consumed last so the
            # previous collective has a full pair of slack
            if k >= 1:
                fillers.append(lambda k=k - 1: recv_unit(k))
            for g in (gA, gB):
                if g < NC_CH:
                    for ii in range(4):
                        fillers.append(
                            lambda g=g, ii=ii: qkv_tile_unit(g, ii))
                    fillers.append(lambda g=g: norm_a_unit(g))
                    fillers.append(lambda g=g: norm_b_unit(g))
            for g in (gA, gB):
                if g < NC_CH:
                    fillers.append(lambda g=g: norm_c_unit(g))
            if k >= 1:
                for dh in range(D // TCH):
                    fillers.append(
                        lambda k=k - 1, dh=dh: proj_unit(k, dh))
            attn_pair(k, fillers)

        # epilogue: last pair's projection (on the freed psy banks so the
        # two output halves don't serialize on the single aux bank)
        recv_unit(NPAIR - 1)
        for dh in range(D // TCH):
            proj_unit(NPAIR - 1, dh, pool=psy)

    nc.compile()
    return nc


def _host_prep(x, ve, qkv_w, lambdas, c_proj_w):
    x = np.asarray(x, dtype=np.float32)
    ve = np.asarray(ve, dtype=np.float32)
    qkv_w = np.asarray(qkv_w, dtype=np.float32)
    lambdas = np.asarray(lambdas, dtype=np.float32)
    c_proj_w = np.asarray(c_proj_w, dtype=np.float32)

    xT = np.ascontiguousarray(x[0].T.astype(NP_MMD))
    cpwT = np.ascontiguousarray(c_proj_w.T.astype(NP_MMD))
    lam_b = np.ascontiguousarray(np.broadcast_to(lambdas, (P, 2)))

    angular = (np.float32(1.0 / 1024.0)
               ** np.linspace(0.0, 1.0, QUARTER, dtype=np.float32))
    t = np.arange(T, dtype=np.float32)
    theta = t[:, None] * angular[None, :]

    def pack(a):  # [T, F] -> [P, NT*F] with row t = tile*128 + p
        F = a.shape[1]
        return np.ascontiguousarray(
            a.reshape(NT, P, F).transpose(1, 0, 2).reshape(P, NT * F)
            .astype(NP_MMD))

    cos_pk = pack(np.cos(theta))
    sin_pk = pack(np.sin(theta))

    in_maps = []
    for h in range(N_CORES):
        sl = slice(h * HD, (h + 1) * HD)
        w_qkvT = np.ascontiguousarray(np.concatenate(
            [qkv_w[0, sl, :].T, qkv_w[1, sl, :].T, qkv_w[2, sl, :].T],
            axis=1).astype(NP_MMD))
        in_maps.append({
            "x_t": xT,
            "w_qkv": w_qkvT,
            "cos_t": cos_pk,
            "sin_t": sin_pk,
            "ve_h": pack(ve[0][:, sl] * lambdas[1]),
            "lam": lam_b,
            "cpw": cpwT,
        })
    return in_maps


def kernel(x, ve, qkv_w, lambdas, c_proj_w, _trace=False, _trace_kwargs=None):
    if "nc" not in _cached:
        _cached["nc"] = build_module()
    nc = _cached["nc"]
    in_maps = _host_prep(x, ve, qkv_w, lambdas, c_proj_w)
    kw = {}
    if _trace:
        kw = dict(trace=True, **(_trace_kwargs or {}))
    res = run_bass_kernel_spmd(nc, in_maps, core_ids=list(range(N_CORES)),
                               **kw)
    _cached["last_result"] = res
    blocks = np.empty((NT, P, D), dtype=np.float32)
    for j in range(N_CORES):
        sh = np.asarray(res.results[j]["y_shard"],
                        dtype=np.float32).reshape(4, P, D)
        for k in range(4):
            blocks[8 * k + j] = sh[k]
    return blocks.reshape(1, T, D)
